# revision 43
# baseline (speedup 1.0000x reference)
"""CrossAttentionFusion Trainium2 kernel.

Problem (per batch element b of 4, C=128 channels, N=4096 tokens):
    Q1 = wq1@hsi+bq1; K1 = wk1@msi+bk1; V1 = wv1@msi+bv1   (1x1 convs)
    Q2 = wq2@msi+bq2; K2 = wk2@hsi+bk2; V2 = wv2@hsi+bv2
    out1 = attn(Q1,K1,V1); out2 = attn(Q2,K2,V2)           (softmax over keys)
    g = sigmoid(wg@[hsi;msi]+bg)
    out = wp@(g*out1 + (1-g)*out2) + bp

Sharding: 8 cores = (b, query-half). Each core computes 2048 query columns
for one batch element; keys/values span all 4096 tokens. Host permutes the
token axis per core so its queries are the first 2048 columns (key order is
irrelevant to attention sums), so the SPMD program is offset-free.

Measured per-op costs (birsim, [128,1024] unless noted):
    ACT exp f32r-out 1113ns (bf16-out 1373 -- avoid); PE 512-row mm f32r
    cadence 232ns (bf16 259); PE 128-row mm bf16 ~120 (f32r ~260, 4x rate
    penalty); DVE TT f32 1220; Pool TT f32 ~2550; DVE TSP ~1100.

Design (ACT-paced at 128 exps + 2 tanh = ~145us/core):
  * One flattened scores->exp->PV pipeline across both attentions x 2
    query-chunks (128 key-tile iterations). PSUM: 3-deep score ring
    (6 banks) + one PV accumulator (2 banks) -- exp never waits.
  * bf16 ONLY for x + conv weights (fast 128-row V^T matmuls, half DMA);
    K/Q/VT/pt and scores/PV matmuls stay f32r (faster on ACT and PE).
  * Softmax denominator: running f32 sums on DVE (m in DVE_MS) and Pool
    (m in POOL_MS), pt[mt-1] left out; then 3 accumulating ones-matmuls
    into a transient ring slot (deferred 2 iterations into the next chunk
    so the PE never waits on the adds). recip reads PSUM directly.
  * p_out is read directly by o = p_out * rec (no eviction copy); the next
    chunk's first PV waits ~4us which the 3-deep ring absorbs.
  * V-bias folds through softmax (weights sum to 1); q/k biases applied at
    conv eviction; o1/o2 biases fold into the gate fusion via
    scalar_tensor_tensor: o2=(o2+bv2)*tb; o1=(o1+bv1)*t; o1+=o2.
    Gate uses sigmoid(z)=0.5*tanh(0.5z)+0.5 with the 0.5 folded into wp.
  * Convs drip into PE slack deadline-driven (emitted 6 iterations before
    first use); all PSUM evictions on DVE (GPSIMD cannot touch PSUM).
"""

import sys

if "/opt/trn_rl_repo" not in sys.path:
    sys.path.insert(0, "/opt/trn_rl_repo")

from contextlib import ExitStack

import numpy as np

import concourse.bacc as bacc
import concourse.bass as bass  # noqa: F401
import concourse.tile as tile
from concourse import mybir

F32 = mybir.dt.float32
F32R = mybir.dt.float32r
BF16 = mybir.dt.bfloat16
C = 128
N_TOK = 4096
NQ = 2048
FD = 512  # matmul moving-operand max
CH = 1024  # query-chunk width (PSUM accumulator width)
SCALE = 1.0 / float(np.sqrt(np.float32(C)))

WEIGHT_NAMES = ["wq1T", "wk1T", "wv1T", "wq2T", "wk2T", "wv2T", "wgaT", "wgbT"]
BIAS_NAMES = ["bq1", "bk1", "bq2", "bk2", "bv1", "bv2", "bgh", "bp"]


def _r(ap):
    return ap.bitcast(F32R)


def build_program(n_tok=N_TOK, nq=NQ, ch=CH, fd=FD):
    mt = n_tok // 128  # key tiles per attention
    nch = nq // ch  # query chunks per attention
    spc = ch // fd  # matmul slices per chunk
    vtg = ch // 128  # V^T tiles per eviction group
    G = 2 * nch * mt  # flattened (attention, chunk, key-tile) iterations

    # denominator ownership: Pool takes ~40% of the tiles (its adds are ~2x
    # slower), DVE the rest; the last tile m=mt-1 goes straight to the
    # ones-matmul so the chunk tail never waits on an add chain.
    pool_ms = {m for m in range(2, mt - 1, 5) for m in (m, m + 2) if m < mt - 1}

    nc = bacc.Bacc("TRN2", target_bir_lowering=False, debug=False)
    din = {}
    for name in ["x_h", "x_m"]:
        din[name] = nc.dram_tensor(name, [C, n_tok], BF16, kind="ExternalInput").ap()
    nw = len(WEIGHT_NAMES)
    din["wpack"] = nc.dram_tensor("wpack", [C, nw * C], BF16, kind="ExternalInput").ap()
    din["wpTs"] = nc.dram_tensor("wpTs", [C, C], F32, kind="ExternalInput").ap()
    din["bpack"] = nc.dram_tensor(
        "bpack", [C, len(BIAS_NAMES)], F32, kind="ExternalInput"
    ).ap()
    out_d = nc.dram_tensor("out", [C, nq], F32, kind="ExternalOutput").ap()

    with ExitStack() as ctx:
        tc = ctx.enter_context(tile.TileContext(nc))
        const = ctx.enter_context(tc.tile_pool(name="const", bufs=1))
        big = ctx.enter_context(tc.tile_pool(name="big", bufs=1))
        ppool = ctx.enter_context(tc.tile_pool(name="ppool", bufs=11))
        tpool = ctx.enter_context(tc.tile_pool(name="tpool", bufs=4))
        dpool = ctx.enter_context(tc.tile_pool(name="dpool", bufs=2))
        stpool = ctx.enter_context(tc.tile_pool(name="stpool", bufs=2))
        ps_pool = ctx.enter_context(tc.tile_pool(name="ps", bufs=2, space="PSUM"))
        pacc_pool = ctx.enter_context(tc.tile_pool(name="pacc", bufs=2, space="PSUM"))

        # constants in
        wpack_sb = const.tile([C, nw * C], BF16, name="wpack")
        nc.sync.dma_start(out=wpack_sb[:], in_=din["wpack"][:])
        bpack_sb = const.tile([C, len(BIAS_NAMES)], F32, name="bpack")
        wp_sb = const.tile([C, C], F32R, name="wpTs")
        w_sb = {
            name: wpack_sb[:, i * C : (i + 1) * C]
            for i, name in enumerate(WEIGHT_NAMES)
        }
        b_sb = {name: bpack_sb[:, i : i + 1] for i, name in enumerate(BIAS_NAMES)}
        ones_sb = const.tile([C, C], F32, name="ones")
        nc.vector.memset(ones_sb[:], 1.0)

        # activations in, chunked (ch columns) across the two HWDGE rings.
        # Only chunk 0 is emitted here; the rest are emitted after the head
        # convs so those convs depend solely on chunk-0 writes (the tile
        # framework orders readers after every write emitted before them).
        xh_sb = big.tile([C, n_tok], BF16, name="xh")
        xm_sb = big.tile([C, n_tok], BF16, name="xm")
        sl0 = slice(0, ch)
        nc.sync.dma_start(out=xm_sb[:, sl0], in_=din["x_m"][:, sl0])
        nc.scalar.dma_start(out=xh_sb[:, sl0], in_=din["x_h"][:, sl0])
        nc.scalar.dma_start(out=bpack_sb[:], in_=din["bpack"][:])
        nc.scalar.dma_start(out=wp_sb[:], in_=_r(din["wpTs"][:]))

        def emit_tail_x_dmas():
            for j in range(1, n_tok // ch):
                sl = slice(j * ch, (j + 1) * ch)
                nc.sync.dma_start(out=xm_sb[:, sl], in_=din["x_m"][:, sl])
                nc.scalar.dma_start(out=xh_sb[:, sl], in_=din["x_h"][:, sl])

        K1_sb = big.tile([C, n_tok], F32R, name="K1")
        K2_sb = big.tile([C, n_tok], F32R, name="K2")
        VT1_sb = big.tile([C, n_tok], F32R, name="VT1")
        VT2_sb = big.tile([C, n_tok], F32R, name="VT2")
        Q1_sb = big.tile([C, nq], F32R, name="Q1")
        Q2_sb = big.tile([C, nq], F32R, name="Q2")
        o1_sb = big.tile([C, nq], F32R, name="o1")
        o2_sb = big.tile([C, nq], BF16, name="o2")
        t_sb = big.tile([C, nq], BF16, name="t")
        tb_sb = big.tile([C, nq], BF16, name="tb")

        # ---- conv thunks (PE matmuls into a ring slot + eviction) ----
        # evictions alternate DVE / ACT: DVE also owns the denominator
        # merges and ACT has slack between exps, so splitting the PSUM
        # drains keeps either queue from backing up
        nev = [0]

        def conv_chunk(dst_sb, wname, x_sb, j, bname):
            def run():
                ps = ps_pool.tile([C, ch], F32, tag="ps", name="cps")
                for s in range(spc):
                    sl = slice(j * ch + s * fd, j * ch + (s + 1) * fd)
                    nc.tensor.matmul(
                        ps[:, s * fd : (s + 1) * fd],
                        w_sb[wname],
                        x_sb[:, sl],
                        start=True,
                        stop=True,
                    )
                dsl = slice(j * ch, (j + 1) * ch)
                nev[0] += 1
                if nev[0] % 2 == 0:
                    nc.scalar.activation(
                        dst_sb[:, dsl], ps[:],
                        mybir.ActivationFunctionType.Identity, bias=b_sb[bname],
                    )
                else:
                    nc.vector.tensor_scalar_add(dst_sb[:, dsl], ps[:], b_sb[bname])

            return run

        def vt_chunk(dst_sb, x_sb, wname, g):
            # dst tile j holds V^T rows for tokens [128j,128j+128): [tok,chan]
            def run():
                ps = ps_pool.tile([C, ch], F32, tag="ps", name="vps")
                for u in range(vtg):
                    j = g * vtg + u
                    nc.tensor.matmul(
                        ps[:, u * 128 : (u + 1) * 128],
                        x_sb[:, j * 128 : (j + 1) * 128],
                        w_sb[wname],
                        start=True,
                        stop=True,
                    )
                nc.scalar.copy(dst_sb[:, g * ch : (g + 1) * ch], ps[:])

            return run

        def gate_chunk(j):
            # t = tanh(0.5*(wgA@xq_h + wgB@xq_m) + 0.5*bg); then
            # t <- 1+t, tb <- 1-t for the 3-op fusion (0.5 folded into wp)
            def run():
                ps = ps_pool.tile([C, ch], F32, tag="ps", name="gps")
                for s in range(spc):
                    sl = slice(j * ch + s * fd, j * ch + (s + 1) * fd)
                    psl = ps[:, s * fd : (s + 1) * fd]
                    nc.tensor.matmul(
                        psl, w_sb["wgaT"], xh_sb[:, sl], start=True, stop=False
                    )
                    nc.tensor.matmul(
                        psl, w_sb["wgbT"], xm_sb[:, sl], start=False, stop=True
                    )
                dsl = slice(j * ch, (j + 1) * ch)
                nc.scalar.activation(
                    t_sb[:, dsl],
                    ps[:],
                    mybir.ActivationFunctionType.Tanh,
                    bias=b_sb["bgh"],
                    scale=0.5,
                )
                nc.vector.tensor_scalar(
                    tb_sb[:, dsl], t_sb[:, dsl], -1.0, 1.0,
                    mybir.AluOpType.mult, mybir.AluOpType.add,
                )
                nc.vector.tensor_scalar_add(t_sb[:, dsl], t_sb[:, dsl], 1.0)

            return run

        # Conv feed with deadlines = last flattened-loop iteration at which
        # the thunk may be emitted (consumer emission iteration minus 1).
        g1, g2 = 0, nch * mt
        feed = []  # (need, x_chunk, thunk); emission aims ~12 early ("want")
        for j in range(n_tok // ch):
            feed.append((g1 + j * vtg - 4, j, conv_chunk(K1_sb, "wk1T", xm_sb, j, "bk1")))
        for j in range(nq // ch):
            feed.append((g1 + j * mt - 4, j, conv_chunk(Q1_sb, "wq1T", xh_sb, j, "bq1")))
        for g in range(mt // vtg):
            feed.append((g1 + g * vtg - 1, g, vt_chunk(VT1_sb, xm_sb, "wv1T", g)))
        for j in range(n_tok // ch):
            feed.append((g2 + j * vtg - 4, j, conv_chunk(K2_sb, "wk2T", xh_sb, j, "bk2")))
        for j in range(nq // ch):
            feed.append((g2 + j * mt - 4, j, conv_chunk(Q2_sb, "wq2T", xm_sb, j, "bq2")))
        for g in range(mt // vtg):
            feed.append((g2 + g * vtg - 1, g, vt_chunk(VT2_sb, xh_sb, "wv2T", g)))
        for j in range(nq // ch):
            feed.append((g2 + (j + 1) * mt - 2, j, gate_chunk(j)))
        feed.sort(key=lambda p: p[0])

        # head: items needed before the loop. Chunk-0-dependent ones go
        # first (only the chunk-0 x DMA is emitted yet), then the remaining
        # x DMAs, then any other pre-loop items (small configs only).
        head0 = [it for it in feed if it[0] < 0 and it[1] == 0]
        headx = [it for it in feed if it[0] < 0 and it[1] != 0]
        feed = [it for it in feed if it[0] >= 0]
        for _, _, th in head0:
            th()
        emit_tail_x_dmas()
        for _, _, th in headx:
            th()

        # ---- flattened attention pipeline -------------------------------
        regions = []
        for cidx in range(nch):
            regions.append((K1_sb, Q1_sb, VT1_sb, o1_sb, False, cidx))
        for cidx in range(nch):
            regions.append((K2_sb, Q2_sb, VT2_sb, o2_sb, True, cidx))

        pts = {}
        pe_q = []  # deferred PE work: denominator matmuls, projections

        def emit_scores(g):
            K_sb, Q_sb, _, _, _, cidx = regions[g // mt]
            m = g % mt
            ksl = slice(m * 128, (m + 1) * 128)
            ps = ps_pool.tile([C, ch], F32, tag="ps", name="sps")
            for s in range(spc):
                qsl = slice(cidx * ch + s * fd, cidx * ch + (s + 1) * fd)
                nc.tensor.matmul(
                    ps[:, s * fd : (s + 1) * fd],
                    K_sb[:, ksl],
                    Q_sb[:, qsl],
                    start=True,
                    stop=True,
                )
            pt = ppool.tile([C, ch], F32R, tag="pt", name="pt")
            nc.scalar.activation(
                pt[:], ps[:], mybir.ActivationFunctionType.Exp, scale=SCALE
            )
            pts[g] = pt

        def den_mm(dps, a, start, stop):
            for s in range(spc):
                ssl = slice(s * fd, (s + 1) * fd)
                nc.tensor.matmul(
                    dps[:, ssl], _r(ones_sb[:]), a[:, ssl], start=start, stop=stop
                )

        def make_tail(o_sb, dps, lates, o_src, cidx, is2, glast):
            # finish the denominator (late ones-matmuls into the ring slot
            # opened at chunk end), recip, o-mul; then (attn2) bias-folded
            # gate fusion + projection, as separate thunks to keep the DVE
            # queue from bursting at chunk boundaries
            def fuse_slice(s):
                def run():
                    sl = slice(cidx * ch + s * fd, cidx * ch + (s + 1) * fd)
                    eng = nc.vector
                    eng.scalar_tensor_tensor(
                        o2_sb[:, sl], o2_sb[:, sl], b_sb["bv2"], tb_sb[:, sl],
                        mybir.AluOpType.add, mybir.AluOpType.mult,
                    )
                    eng.scalar_tensor_tensor(
                        o1_sb[:, sl], o1_sb[:, sl], b_sb["bv1"], t_sb[:, sl],
                        mybir.AluOpType.add, mybir.AluOpType.mult,
                    )
                    eng.tensor_add(o1_sb[:, sl], o1_sb[:, sl], o2_sb[:, sl])
                    pe_q.append(make_proj(sl))

                return run

            def run():
                for i, a in enumerate(lates):
                    den_mm(dps, a, False, i == len(lates) - 1)
                rec = dpool.tile([C, ch], F32, tag="rec", name="rec")
                if glast and is2:
                    # final chunk: 512-wide pipelined epilogue so the first
                    # output DMA starts as early as possible
                    for s in range(spc):
                        ssl = slice(s * fd, (s + 1) * fd)
                        osl = slice(cidx * ch + s * fd, cidx * ch + (s + 1) * fd)
                        nc.vector.reciprocal_approx_fast(rec[:, ssl], dps[:, ssl])
                        nc.vector.tensor_mul(o_sb[:, osl], o_src[:, ssl], rec[:, ssl])
                        fuse_slice(s)()
                        pe_q.pop(0)()  # the projection this fuse enqueued
                    while pe_q:
                        pe_q.pop(0)()
                    return
                nc.vector.reciprocal_approx_fast(rec[:], dps[:])
                osl = slice(cidx * ch, (cidx + 1) * ch)
                nc.vector.tensor_mul(o_sb[:, osl], o_src[:], rec[:])
                if is2:
                    for s in range(spc):
                        pe_q.append(fuse_slice(s))

            return run

        def make_proj(sl):
            def run():
                ps = ps_pool.tile([C, fd], F32, tag="ps", name="pjps")
                nc.tensor.matmul(ps[:], wp_sb[:], o1_sb[:, sl], start=True, stop=True)
                st = stpool.tile([C, fd], F32, tag="st", name="st")
                nc.vector.tensor_scalar_add(st[:], ps[:], b_sb["bp"])
                nc.sync.dma_start(out=out_d[:, sl], in_=st[:])

            return run

        emit_scores(0)
        emit_scores(1)

        p_out = None
        avail = []  # denominator merge-tree: summed-subtree tiles, oldest first
        n_tree = [0]  # merge ops emitted this chunk (for DVE/Pool balancing)
        dps_cur = None
        den_started = [False]

        def tree_merge(limit):
            # Merge the two oldest available subtree tiles while more than
            # `limit` remain. Ops touch distinct tiles so they pipeline
            # cleanly (serial accumulate chains measure ~2.6x slower).
            while len(avail) > limit:
                a = avail.pop(0)
                b = avail.pop(0)
                node = tpool.tile([C, ch], F32R, tag="tn", name="tn", bufs=5)
                # Pool takes 1 of every 3 merges (its adds are ~2x slower)
                if n_tree[0] % 3 == 1:
                    nc.gpsimd.tensor_add(node[:], a[:], b[:])
                else:
                    nc.vector.tensor_add(node[:], a[:], b[:])
                n_tree[0] += 1
                avail.append(node)

        for g in range(G):
            K_sb, Q_sb, VT_sb, o_sb, is2, cidx = regions[g // mt]
            m = g % mt
            if g + 2 < G:
                emit_scores(g + 2)

            # PV accumulation for key-tile m
            if m == 0:
                p_out = pacc_pool.tile([C, ch], F32, tag="acc", name="pacc")
                avail = []
                n_tree[0] = 0
            ksl = slice(m * 128, (m + 1) * 128)
            pt = pts[g]
            for s in range(spc):
                ssl = slice(s * fd, (s + 1) * fd)
                nc.tensor.matmul(
                    p_out[:, ssl], VT_sb[:, ksl], pt[:, ssl],
                    start=m == 0, stop=m == mt - 1,
                )

            # denominator merge tree; the trailing tiles go straight to
            # the ones-matmuls (4 of them in the final chunk so recip can
            # start right after the last exp, 1 otherwise). The subtree
            # matmuls wait on DVE/Pool merges, so for mid-kernel chunks they
            # are deferred into the next chunk (make_tail) -- only pt[mt-1]'s
            # matmul is inline (it waits just its exp).
            glast_chunk = g // mt == len(regions) - 1
            ndirect = min(8, mt) if glast_chunk else min(2, mt)
            if m < mt - ndirect:
                avail.append(pts.pop(g))
                tree_merge(4)
            else:
                if m == mt - ndirect:
                    dps_cur = ps_pool.tile([C, ch], F32, tag="ps", name="dps")
                    den_started[0] = False
                # the pt matmul first (waits only its exp); merge-dependent
                # subtree matmuls after so they never block a ready pt
                if m < mt - 1:
                    den_mm(dps_cur, pts.pop(g), not den_started[0], False)
                    den_started[0] = True
                    if glast_chunk and avail:
                        den_mm(dps_cur, avail.pop(0), False, False)
                else:
                    fin = not avail if glast_chunk else False
                    den_mm(dps_cur, pts.pop(g), not den_started[0], fin)
                    den_started[0] = True
                    if glast_chunk:
                        while avail:
                            den_mm(dps_cur, avail.pop(0), False, not avail)

            # conv feed: forced by the correctness deadline, else one per
            # iteration up to ~20 early (the DVE eviction sits behind queued
            # work, so it must enter the queue well before the data is read)
            while feed and feed[0][0] <= g + 1:
                feed.pop(0)[2]()
            if feed and feed[0][0] <= g + 20:
                feed.pop(0)[2]()
            if pe_q and m in (2, 4, 6, 8, 10, 12, 14):
                pe_q.pop(0)()

            if m == mt - 1:
                glast = g == G - 1
                lates = list(avail)
                avail = []
                if glast:
                    # no next chunk contends for the accumulator: read PSUM
                    make_tail(o_sb, dps_cur, [], p_out, cidx, is2, True)()
                else:
                    # pacc has 2 buffers: the deferred tail reads this
                    # chunk's accumulator while the next chunk fills the other
                    pe_q.insert(
                        0,
                        make_tail(o_sb, dps_cur, lates, p_out, cidx, is2, False),
                    )

    nc.compile()
    return nc


def make_in_maps(hsi, msi, weights, n_cores=8):
    """Host-side sharding: core i handles (b=i//2, half=i%2); the token axis is
    rotated so the core's queries are columns [0, NQ). Inputs cast to bf16."""
    import ml_dtypes

    B = hsi.shape[0]
    hsi = np.asarray(hsi, np.float32).reshape(B, C, N_TOK).astype(ml_dtypes.bfloat16)
    msi = np.asarray(msi, np.float32).reshape(B, C, N_TOK).astype(ml_dtypes.bfloat16)
    in_maps = []
    for core in range(n_cores):
        b, h = core // 2, core % 2
        if h == 0:
            x_h, x_m = hsi[b], msi[b]
        else:
            x_h = np.concatenate([hsi[b][:, NQ:], hsi[b][:, :NQ]], axis=1)
            x_m = np.concatenate([msi[b][:, NQ:], msi[b][:, :NQ]], axis=1)
        m = {"x_h": np.ascontiguousarray(x_h), "x_m": np.ascontiguousarray(x_m)}
        m.update(weights)
        in_maps.append(m)
    return in_maps


def make_weight_map(
    wq1, bq1, wk1, bk1, wv1, bv1, wq2, bq2, wk2, bk2, wv2, bv2, wg, bg, wp, bp
):
    import ml_dtypes

    f = np.float32
    col = lambda v: np.ascontiguousarray(np.asarray(v, f).reshape(C, 1))
    tr = lambda w: np.ascontiguousarray(np.asarray(w, f).T)
    w = {
        "wq1T": tr(wq1), "wk1T": tr(wk1), "wv1T": tr(wv1),
        "wq2T": tr(wq2), "wk2T": tr(wk2), "wv2T": tr(wv2),
        "wgaT": tr(np.asarray(wg, f)[:, :C]),
        "wgbT": tr(np.asarray(wg, f)[:, C:]),
    }
    b = {
        "bq1": col(bq1), "bk1": col(bk1), "bq2": col(bq2), "bk2": col(bk2),
        "bv1": col(bv1), "bv2": col(bv2), "bgh": col(0.5 * np.asarray(bg, f)),
        "bp": col(bp),
    }
    wpack = np.concatenate([w[n] for n in WEIGHT_NAMES], axis=1)
    bpack = np.concatenate([b[n] for n in BIAS_NAMES], axis=1)
    return {
        "wpack": np.ascontiguousarray(wpack.astype(ml_dtypes.bfloat16)),
        "wpTs": np.ascontiguousarray(tr(0.5 * np.asarray(wp, f))),
        "bpack": np.ascontiguousarray(bpack),
    }


_NC_CACHE = {}


def _get_program():
    if "nc" not in _NC_CACHE:
        _NC_CACHE["nc"] = build_program()
    return _NC_CACHE["nc"]


def run_on_cores(in_maps, trace=False, **kwargs):
    from concourse.bass_utils import run_bass_kernel_spmd

    nc = _get_program()
    return run_bass_kernel_spmd(
        nc, in_maps, core_ids=list(range(len(in_maps))), trace=trace, **kwargs
    )


def kernel(
    hsi, msi, wq1, bq1, wk1, bk1, wv1, bv1, wq2, bq2, wk2, bk2, wv2, bv2,
    wg, bg, wp, bp,
):
    B, _, H, W = hsi.shape
    weights = make_weight_map(
        wq1, bq1, wk1, bk1, wv1, bv1, wq2, bq2, wk2, bk2, wv2, bv2, wg, bg, wp, bp
    )
    in_maps = make_in_maps(np.asarray(hsi), np.asarray(msi), weights)
    res = run_on_cores(in_maps)
    out = np.zeros((B, C, N_TOK), dtype=np.float32)
    for core in range(8):
        b, h = core // 2, core % 2
        out[b][:, h * NQ : (h + 1) * NQ] = res.results[core]["out"]
    return out.reshape(B, C, N_TOK // 64, 64)


# revision 44
# speedup vs baseline: 1.0547x; 1.0547x over previous
"""CrossAttentionFusion Trainium2 kernel.

Problem (per batch element b of 4, C=128 channels, N=4096 tokens):
    Q1 = wq1@hsi+bq1; K1 = wk1@msi+bk1; V1 = wv1@msi+bv1   (1x1 convs)
    Q2 = wq2@msi+bq2; K2 = wk2@hsi+bk2; V2 = wv2@hsi+bv2
    out1 = attn(Q1,K1,V1); out2 = attn(Q2,K2,V2)           (softmax over keys)
    g = sigmoid(wg@[hsi;msi]+bg)
    out = wp@(g*out1 + (1-g)*out2) + bp

Sharding: 8 cores = (b, query-half). Each core computes 2048 query columns
for one batch element; keys/values span all 4096 tokens. Host permutes the
token axis per core so its queries are the first 2048 columns (key order is
irrelevant to attention sums), so the SPMD program is offset-free.

Measured per-op costs (birsim, [128,1024] unless noted):
    ACT exp f32r-out 1113ns (bf16-out 1373 -- avoid); PE 512-row mm f32r
    cadence 232ns (bf16 259); PE 128-row mm bf16 ~120 (f32r ~260, 4x rate
    penalty); DVE TT f32 1220; Pool TT f32 ~2550; DVE TSP ~1100.

Design (ACT-paced at 128 exps + 2 tanh = ~145us/core):
  * One flattened scores->exp->PV pipeline across both attentions x 2
    query-chunks (128 key-tile iterations). PSUM: 3-deep score ring
    (6 banks) + one PV accumulator (2 banks) -- exp never waits.
  * bf16 ONLY for x + conv weights (fast 128-row V^T matmuls, half DMA);
    K/Q/VT/pt and scores/PV matmuls stay f32r (faster on ACT and PE).
  * Softmax denominator: running f32 sums on DVE (m in DVE_MS) and Pool
    (m in POOL_MS), pt[mt-1] left out; then 3 accumulating ones-matmuls
    into a transient ring slot (deferred 2 iterations into the next chunk
    so the PE never waits on the adds). recip reads PSUM directly.
  * p_out is read directly by o = p_out * rec (no eviction copy); the next
    chunk's first PV waits ~4us which the 3-deep ring absorbs.
  * V-bias folds through softmax (weights sum to 1); q/k biases applied at
    conv eviction; o1/o2 biases fold into the gate fusion via
    scalar_tensor_tensor: o2=(o2+bv2)*tb; o1=(o1+bv1)*t; o1+=o2.
    Gate uses sigmoid(z)=0.5*tanh(0.5z)+0.5 with the 0.5 folded into wp.
  * Convs drip into PE slack deadline-driven (emitted 6 iterations before
    first use); all PSUM evictions on DVE (GPSIMD cannot touch PSUM).
"""

import sys

if "/opt/trn_rl_repo" not in sys.path:
    sys.path.insert(0, "/opt/trn_rl_repo")

from contextlib import ExitStack

import numpy as np

import concourse.bacc as bacc
import concourse.bass as bass  # noqa: F401
import concourse.tile as tile
from concourse import mybir

F32 = mybir.dt.float32
F32R = mybir.dt.float32r
BF16 = mybir.dt.bfloat16
C = 128
N_TOK = 4096
NQ = 2048
FD = 512  # matmul moving-operand max
CH = 1024  # query-chunk width (PSUM accumulator width)
SCALE = 1.0 / float(np.sqrt(np.float32(C)))

WEIGHT_NAMES = ["wq1T", "wk1T", "wv1T", "wq2T", "wk2T", "wv2T", "wgaT", "wgbT"]
BIAS_NAMES = ["bq1", "bk1", "bq2", "bk2", "bv1", "bv2", "bgh", "bp"]


def _r(ap):
    return ap.bitcast(F32R)


def build_program(n_tok=N_TOK, nq=NQ, ch=CH, fd=FD):
    mt = n_tok // 128  # key tiles per attention
    nch = nq // ch  # query chunks per attention
    spc = ch // fd  # matmul slices per chunk
    vtg = ch // 128  # V^T tiles per eviction group
    G = 2 * nch * mt  # flattened (attention, chunk, key-tile) iterations

    # denominator ownership: Pool takes ~40% of the tiles (its adds are ~2x
    # slower), DVE the rest; the last tile m=mt-1 goes straight to the
    # ones-matmul so the chunk tail never waits on an add chain.
    pool_ms = {m for m in range(2, mt - 1, 5) for m in (m, m + 2) if m < mt - 1}

    nc = bacc.Bacc("TRN2", target_bir_lowering=False, debug=False)
    din = {}
    for name in ["x_h", "x_m"]:
        din[name] = nc.dram_tensor(name, [C, n_tok], BF16, kind="ExternalInput").ap()
    nw = len(WEIGHT_NAMES)
    din["wpack"] = nc.dram_tensor("wpack", [C, nw * C], BF16, kind="ExternalInput").ap()
    din["wpTs"] = nc.dram_tensor("wpTs", [C, C], F32, kind="ExternalInput").ap()
    din["bpack"] = nc.dram_tensor(
        "bpack", [C, len(BIAS_NAMES)], F32, kind="ExternalInput"
    ).ap()
    out_d = nc.dram_tensor("out", [C, nq], F32, kind="ExternalOutput").ap()

    with ExitStack() as ctx:
        tc = ctx.enter_context(tile.TileContext(nc))
        const = ctx.enter_context(tc.tile_pool(name="const", bufs=1))
        big = ctx.enter_context(tc.tile_pool(name="big", bufs=1))
        ppool = ctx.enter_context(tc.tile_pool(name="ppool", bufs=11))
        tpool = ctx.enter_context(tc.tile_pool(name="tpool", bufs=4))
        dpool = ctx.enter_context(tc.tile_pool(name="dpool", bufs=2))
        stpool = ctx.enter_context(tc.tile_pool(name="stpool", bufs=2))
        ps_pool = ctx.enter_context(tc.tile_pool(name="ps", bufs=3, space="PSUM"))
        pacc_pool = ctx.enter_context(tc.tile_pool(name="pacc", bufs=1, space="PSUM"))

        # constants in
        wpack_sb = const.tile([C, nw * C], BF16, name="wpack")
        nc.sync.dma_start(out=wpack_sb[:], in_=din["wpack"][:])
        bpack_sb = const.tile([C, len(BIAS_NAMES)], F32, name="bpack")
        wp_sb = const.tile([C, C], F32R, name="wpTs")
        w_sb = {
            name: wpack_sb[:, i * C : (i + 1) * C]
            for i, name in enumerate(WEIGHT_NAMES)
        }
        b_sb = {name: bpack_sb[:, i : i + 1] for i, name in enumerate(BIAS_NAMES)}
        ones_sb = const.tile([C, C], F32, name="ones")
        nc.vector.memset(ones_sb[:], 1.0)

        # activations in, chunked (ch columns) across the two HWDGE rings.
        # Only chunk 0 is emitted here; the rest are emitted after the head
        # convs so those convs depend solely on chunk-0 writes (the tile
        # framework orders readers after every write emitted before them).
        xh_sb = big.tile([C, n_tok], BF16, name="xh")
        xm_sb = big.tile([C, n_tok], BF16, name="xm")
        sl0 = slice(0, ch)
        nc.sync.dma_start(out=xm_sb[:, sl0], in_=din["x_m"][:, sl0])
        nc.scalar.dma_start(out=xh_sb[:, sl0], in_=din["x_h"][:, sl0])
        nc.scalar.dma_start(out=bpack_sb[:], in_=din["bpack"][:])
        nc.scalar.dma_start(out=wp_sb[:], in_=_r(din["wpTs"][:]))

        def emit_tail_x_dmas():
            for j in range(1, n_tok // ch):
                sl = slice(j * ch, (j + 1) * ch)
                nc.sync.dma_start(out=xm_sb[:, sl], in_=din["x_m"][:, sl])
                nc.scalar.dma_start(out=xh_sb[:, sl], in_=din["x_h"][:, sl])

        K1_sb = big.tile([C, n_tok], F32R, name="K1")
        K2_sb = big.tile([C, n_tok], F32R, name="K2")
        VT1_sb = big.tile([C, n_tok], F32R, name="VT1")
        VT2_sb = big.tile([C, n_tok], F32R, name="VT2")
        Q1_sb = big.tile([C, nq], F32R, name="Q1")
        Q2_sb = big.tile([C, nq], F32R, name="Q2")
        o1_sb = big.tile([C, nq], F32R, name="o1")
        o2_sb = big.tile([C, nq], BF16, name="o2")
        t_sb = big.tile([C, nq], BF16, name="t")
        tb_sb = big.tile([C, nq], BF16, name="tb")

        # ---- conv thunks (PE matmuls into a ring slot + eviction) ----
        # evictions alternate DVE / ACT: DVE also owns the denominator
        # merges and ACT has slack between exps, so splitting the PSUM
        # drains keeps either queue from backing up
        nev = [0]

        def conv_chunk(dst_sb, wname, x_sb, j, bname):
            def run():
                ps = ps_pool.tile([C, ch], F32, tag="ps", name="cps")
                for s in range(spc):
                    sl = slice(j * ch + s * fd, j * ch + (s + 1) * fd)
                    nc.tensor.matmul(
                        ps[:, s * fd : (s + 1) * fd],
                        w_sb[wname],
                        x_sb[:, sl],
                        start=True,
                        stop=True,
                    )
                dsl = slice(j * ch, (j + 1) * ch)
                nev[0] += 1
                if nev[0] % 2 == 0:
                    nc.scalar.activation(
                        dst_sb[:, dsl], ps[:],
                        mybir.ActivationFunctionType.Identity, bias=b_sb[bname],
                    )
                else:
                    nc.vector.tensor_scalar_add(dst_sb[:, dsl], ps[:], b_sb[bname])

            return run

        def vt_chunk(dst_sb, x_sb, wname, g):
            # dst tile j holds V^T rows for tokens [128j,128j+128): [tok,chan]
            def run():
                ps = ps_pool.tile([C, ch], F32, tag="ps", name="vps")
                for u in range(vtg):
                    j = g * vtg + u
                    nc.tensor.matmul(
                        ps[:, u * 128 : (u + 1) * 128],
                        x_sb[:, j * 128 : (j + 1) * 128],
                        w_sb[wname],
                        start=True,
                        stop=True,
                    )
                nc.scalar.copy(dst_sb[:, g * ch : (g + 1) * ch], ps[:])

            return run

        def gate_chunk(j):
            # t = tanh(0.5*(wgA@xq_h + wgB@xq_m) + 0.5*bg); then
            # t <- 1+t, tb <- 1-t for the 3-op fusion (0.5 folded into wp)
            def run():
                ps = ps_pool.tile([C, ch], F32, tag="ps", name="gps")
                for s in range(spc):
                    sl = slice(j * ch + s * fd, j * ch + (s + 1) * fd)
                    psl = ps[:, s * fd : (s + 1) * fd]
                    nc.tensor.matmul(
                        psl, w_sb["wgaT"], xh_sb[:, sl], start=True, stop=False
                    )
                    nc.tensor.matmul(
                        psl, w_sb["wgbT"], xm_sb[:, sl], start=False, stop=True
                    )
                dsl = slice(j * ch, (j + 1) * ch)
                nc.scalar.activation(
                    t_sb[:, dsl],
                    ps[:],
                    mybir.ActivationFunctionType.Tanh,
                    bias=b_sb["bgh"],
                    scale=0.5,
                )
                nc.vector.tensor_scalar(
                    tb_sb[:, dsl], t_sb[:, dsl], -1.0, 1.0,
                    mybir.AluOpType.mult, mybir.AluOpType.add,
                )
                nc.vector.tensor_scalar_add(t_sb[:, dsl], t_sb[:, dsl], 1.0)

            return run

        # Conv feed with deadlines = last flattened-loop iteration at which
        # the thunk may be emitted (consumer emission iteration minus 1).
        g1, g2 = 0, nch * mt
        feed = []  # (need, x_chunk, thunk); emission aims ~12 early ("want")
        for j in range(n_tok // ch):
            feed.append((g1 + j * vtg - 4, j, conv_chunk(K1_sb, "wk1T", xm_sb, j, "bk1")))
        for j in range(nq // ch):
            feed.append((g1 + j * mt - 4, j, conv_chunk(Q1_sb, "wq1T", xh_sb, j, "bq1")))
        for g in range(mt // vtg):
            feed.append((g1 + g * vtg - 1, g, vt_chunk(VT1_sb, xm_sb, "wv1T", g)))
        for j in range(n_tok // ch):
            feed.append((g2 + j * vtg - 4, j, conv_chunk(K2_sb, "wk2T", xh_sb, j, "bk2")))
        for j in range(nq // ch):
            feed.append((g2 + j * mt - 4, j, conv_chunk(Q2_sb, "wq2T", xm_sb, j, "bq2")))
        for g in range(mt // vtg):
            feed.append((g2 + g * vtg - 1, g, vt_chunk(VT2_sb, xh_sb, "wv2T", g)))
        for j in range(nq // ch):
            feed.append((g2 + (j + 1) * mt - 2, j, gate_chunk(j)))
        feed.sort(key=lambda p: p[0])

        # head: items needed before the loop. Chunk-0-dependent ones go
        # first (only the chunk-0 x DMA is emitted yet), then the remaining
        # x DMAs, then any other pre-loop items (small configs only).
        head0 = [it for it in feed if it[0] < 0 and it[1] == 0]
        headx = [it for it in feed if it[0] < 0 and it[1] != 0]
        feed = [it for it in feed if it[0] >= 0]
        for _, _, th in head0:
            th()
        emit_tail_x_dmas()
        for _, _, th in headx:
            th()

        # ---- flattened attention pipeline -------------------------------
        regions = []
        for cidx in range(nch):
            regions.append((K1_sb, Q1_sb, VT1_sb, o1_sb, False, cidx))
        for cidx in range(nch):
            regions.append((K2_sb, Q2_sb, VT2_sb, o2_sb, True, cidx))

        pts = {}
        pe_q = []  # deferred PE work: denominator matmuls, projections

        def emit_scores(g):
            K_sb, Q_sb, _, _, _, cidx = regions[g // mt]
            m = g % mt
            ksl = slice(m * 128, (m + 1) * 128)
            ps = ps_pool.tile([C, ch], F32, tag="ps", name="sps")
            for s in range(spc):
                qsl = slice(cidx * ch + s * fd, cidx * ch + (s + 1) * fd)
                nc.tensor.matmul(
                    ps[:, s * fd : (s + 1) * fd],
                    K_sb[:, ksl],
                    Q_sb[:, qsl],
                    start=True,
                    stop=True,
                )
            pt = ppool.tile([C, ch], F32R, tag="pt", name="pt")
            nc.scalar.activation(
                pt[:], ps[:], mybir.ActivationFunctionType.Exp, scale=SCALE
            )
            pts[g] = pt

        def den_mm(dps, a, start, stop):
            for s in range(spc):
                ssl = slice(s * fd, (s + 1) * fd)
                nc.tensor.matmul(
                    dps[:, ssl], _r(ones_sb[:]), a[:, ssl], start=start, stop=stop
                )

        def make_tail(o_sb, dps, lates, o_src, cidx, is2, glast):
            # finish the denominator (late ones-matmuls into the ring slot
            # opened at chunk end), recip, o-mul; then (attn2) bias-folded
            # gate fusion + projection, as separate thunks to keep the DVE
            # queue from bursting at chunk boundaries
            def fuse_slice(s):
                def run():
                    sl = slice(cidx * ch + s * fd, cidx * ch + (s + 1) * fd)
                    eng = nc.vector
                    eng.scalar_tensor_tensor(
                        o2_sb[:, sl], o2_sb[:, sl], b_sb["bv2"], tb_sb[:, sl],
                        mybir.AluOpType.add, mybir.AluOpType.mult,
                    )
                    eng.scalar_tensor_tensor(
                        o1_sb[:, sl], o1_sb[:, sl], b_sb["bv1"], t_sb[:, sl],
                        mybir.AluOpType.add, mybir.AluOpType.mult,
                    )
                    eng.tensor_add(o1_sb[:, sl], o1_sb[:, sl], o2_sb[:, sl])
                    pe_q.append(make_proj(sl))

                return run

            def run():
                for i, a in enumerate(lates):
                    den_mm(dps, a, False, i == len(lates) - 1)
                rec = dpool.tile([C, ch], F32, tag="rec", name="rec")
                if glast and is2:
                    # final chunk: 512-wide pipelined epilogue so the first
                    # output DMA starts as early as possible
                    for s in range(spc):
                        ssl = slice(s * fd, (s + 1) * fd)
                        osl = slice(cidx * ch + s * fd, cidx * ch + (s + 1) * fd)
                        nc.vector.reciprocal_approx_fast(rec[:, ssl], dps[:, ssl])
                        nc.vector.tensor_mul(o_sb[:, osl], o_src[:, ssl], rec[:, ssl])
                        fuse_slice(s)()
                        pe_q.pop(0)()  # the projection this fuse enqueued
                    while pe_q:
                        pe_q.pop(0)()
                    return
                nc.vector.reciprocal_approx_fast(rec[:], dps[:])
                osl = slice(cidx * ch, (cidx + 1) * ch)
                nc.vector.tensor_mul(o_sb[:, osl], o_src[:], rec[:])
                if is2:
                    for s in range(spc):
                        pe_q.append(fuse_slice(s))

            return run

        def make_proj(sl):
            def run():
                ps = ps_pool.tile([C, fd], F32, tag="ps", name="pjps")
                nc.tensor.matmul(ps[:], wp_sb[:], o1_sb[:, sl], start=True, stop=True)
                st = stpool.tile([C, fd], F32, tag="st", name="st")
                nc.vector.tensor_scalar_add(st[:], ps[:], b_sb["bp"])
                nc.sync.dma_start(out=out_d[:, sl], in_=st[:])

            return run

        emit_scores(0)
        emit_scores(1)
        emit_scores(2)

        p_out = None
        avail = []  # denominator merge-tree: summed-subtree tiles, oldest first
        n_tree = [0]  # merge ops emitted this chunk (for DVE/Pool balancing)
        dps_cur = None
        den_started = [False]

        def tree_merge(limit):
            # Merge the two oldest available subtree tiles while more than
            # `limit` remain. Ops touch distinct tiles so they pipeline
            # cleanly (serial accumulate chains measure ~2.6x slower).
            while len(avail) > limit:
                a = avail.pop(0)
                b = avail.pop(0)
                node = tpool.tile([C, ch], F32R, tag="tn", name="tn", bufs=5)
                # Pool takes 1 of every 3 merges (its adds are ~2x slower)
                if n_tree[0] % 3 == 1:
                    nc.gpsimd.tensor_add(node[:], a[:], b[:])
                else:
                    nc.vector.tensor_add(node[:], a[:], b[:])
                n_tree[0] += 1
                avail.append(node)

        for g in range(G):
            K_sb, Q_sb, VT_sb, o_sb, is2, cidx = regions[g // mt]
            m = g % mt
            if g + 3 < G:
                emit_scores(g + 3)

            # PV accumulation for key-tile m
            if m == 0:
                p_out = pacc_pool.tile([C, ch], F32, tag="acc", name="pacc")
                avail = []
                n_tree[0] = 0
            ksl = slice(m * 128, (m + 1) * 128)
            pt = pts[g]
            for s in range(spc):
                ssl = slice(s * fd, (s + 1) * fd)
                nc.tensor.matmul(
                    p_out[:, ssl], VT_sb[:, ksl], pt[:, ssl],
                    start=m == 0, stop=m == mt - 1,
                )

            # denominator merge tree; the trailing tiles go straight to
            # the ones-matmuls (4 of them in the final chunk so recip can
            # start right after the last exp, 1 otherwise). The subtree
            # matmuls wait on DVE/Pool merges, so for mid-kernel chunks they
            # are deferred into the next chunk (make_tail) -- only pt[mt-1]'s
            # matmul is inline (it waits just its exp).
            glast_chunk = g // mt == len(regions) - 1
            ndirect = min(8, mt) if glast_chunk else min(2, mt)
            if m < mt - ndirect:
                avail.append(pts.pop(g))
                tree_merge(4)
            else:
                if m == mt - ndirect:
                    dps_cur = ps_pool.tile([C, ch], F32, tag="ps", name="dps")
                    den_started[0] = False
                # the pt matmul first (waits only its exp); merge-dependent
                # subtree matmuls after so they never block a ready pt
                if m < mt - 1:
                    den_mm(dps_cur, pts.pop(g), not den_started[0], False)
                    den_started[0] = True
                    if glast_chunk and avail:
                        den_mm(dps_cur, avail.pop(0), False, False)
                else:
                    fin = not avail if glast_chunk else False
                    den_mm(dps_cur, pts.pop(g), not den_started[0], fin)
                    den_started[0] = True
                    if glast_chunk:
                        while avail:
                            den_mm(dps_cur, avail.pop(0), False, not avail)

            # conv feed: forced by the correctness deadline, else one per
            # iteration up to ~20 early (the DVE eviction sits behind queued
            # work, so it must enter the queue well before the data is read)
            while feed and feed[0][0] <= g + 1:
                feed.pop(0)[2]()
            if feed and feed[0][0] <= g + 20:
                feed.pop(0)[2]()
            if pe_q and m in (2, 4, 6, 8, 10, 12, 14):
                pe_q.pop(0)()

            if m == mt - 1:
                glast = g == G - 1
                lates = list(avail)
                avail = []
                if glast:
                    # no next chunk contends for the accumulator: read PSUM
                    make_tail(o_sb, dps_cur, [], p_out, cidx, is2, True)()
                else:
                    # evict p_out NOW (before the next chunk's PV reuses the
                    # single PSUM accumulator buffer); defer the rest so the
                    # PE stream never waits on the remaining adds
                    o_raw = tpool.tile([C, ch], F32R, tag="oraw", name="oraw", bufs=2)
                    nc.vector.tensor_copy(o_raw[:], p_out[:])
                    pe_q.insert(
                        0,
                        make_tail(o_sb, dps_cur, lates, o_raw, cidx, is2, False),
                    )

    nc.compile()
    return nc


def make_in_maps(hsi, msi, weights, n_cores=8):
    """Host-side sharding: core i handles (b=i//2, half=i%2); the token axis is
    rotated so the core's queries are columns [0, NQ). Inputs cast to bf16."""
    import ml_dtypes

    B = hsi.shape[0]
    hsi = np.asarray(hsi, np.float32).reshape(B, C, N_TOK).astype(ml_dtypes.bfloat16)
    msi = np.asarray(msi, np.float32).reshape(B, C, N_TOK).astype(ml_dtypes.bfloat16)
    in_maps = []
    for core in range(n_cores):
        b, h = core // 2, core % 2
        if h == 0:
            x_h, x_m = hsi[b], msi[b]
        else:
            x_h = np.concatenate([hsi[b][:, NQ:], hsi[b][:, :NQ]], axis=1)
            x_m = np.concatenate([msi[b][:, NQ:], msi[b][:, :NQ]], axis=1)
        m = {"x_h": np.ascontiguousarray(x_h), "x_m": np.ascontiguousarray(x_m)}
        m.update(weights)
        in_maps.append(m)
    return in_maps


def make_weight_map(
    wq1, bq1, wk1, bk1, wv1, bv1, wq2, bq2, wk2, bk2, wv2, bv2, wg, bg, wp, bp
):
    import ml_dtypes

    f = np.float32
    col = lambda v: np.ascontiguousarray(np.asarray(v, f).reshape(C, 1))
    tr = lambda w: np.ascontiguousarray(np.asarray(w, f).T)
    w = {
        "wq1T": tr(wq1), "wk1T": tr(wk1), "wv1T": tr(wv1),
        "wq2T": tr(wq2), "wk2T": tr(wk2), "wv2T": tr(wv2),
        "wgaT": tr(np.asarray(wg, f)[:, :C]),
        "wgbT": tr(np.asarray(wg, f)[:, C:]),
    }
    b = {
        "bq1": col(bq1), "bk1": col(bk1), "bq2": col(bq2), "bk2": col(bk2),
        "bv1": col(bv1), "bv2": col(bv2), "bgh": col(0.5 * np.asarray(bg, f)),
        "bp": col(bp),
    }
    wpack = np.concatenate([w[n] for n in WEIGHT_NAMES], axis=1)
    bpack = np.concatenate([b[n] for n in BIAS_NAMES], axis=1)
    return {
        "wpack": np.ascontiguousarray(wpack.astype(ml_dtypes.bfloat16)),
        "wpTs": np.ascontiguousarray(tr(0.5 * np.asarray(wp, f))),
        "bpack": np.ascontiguousarray(bpack),
    }


_NC_CACHE = {}


def _get_program():
    if "nc" not in _NC_CACHE:
        _NC_CACHE["nc"] = build_program()
    return _NC_CACHE["nc"]


def run_on_cores(in_maps, trace=False, **kwargs):
    from concourse.bass_utils import run_bass_kernel_spmd

    nc = _get_program()
    return run_bass_kernel_spmd(
        nc, in_maps, core_ids=list(range(len(in_maps))), trace=trace, **kwargs
    )


def kernel(
    hsi, msi, wq1, bq1, wk1, bk1, wv1, bv1, wq2, bq2, wk2, bk2, wv2, bv2,
    wg, bg, wp, bp,
):
    B, _, H, W = hsi.shape
    weights = make_weight_map(
        wq1, bq1, wk1, bk1, wv1, bv1, wq2, bq2, wk2, bk2, wv2, bv2, wg, bg, wp, bp
    )
    in_maps = make_in_maps(np.asarray(hsi), np.asarray(msi), weights)
    res = run_on_cores(in_maps)
    out = np.zeros((B, C, N_TOK), dtype=np.float32)
    for core in range(8):
        b, h = core // 2, core % 2
        out[b][:, h * NQ : (h + 1) * NQ] = res.results[core]["out"]
    return out.reshape(B, C, N_TOK // 64, 64)
